# revision 1
# baseline (speedup 1.0000x reference)
"""Causal self-attention (B=4, T=2048, C=1024, H=16) on 8 trn2 NeuronCores.

Sharding: head-parallel tensor parallelism. Each core owns 2 of the 16 heads:
 - QKV projection computed for its 384 rows of Wqkv (2 heads x 64 x {q,k,v})
 - attention for its 2 heads (causal, block-skipped)
 - partial out-projection against its 128 columns of Wout
The 8 partial [C, B*T] outputs are summed on the host (the "all-reduce").

Device layouts (chosen so every matmul contraction dim lands on SBUF
partitions with zero on-device transposes except a cheap 128x128 PE
transpose of V):
  xT    [C, B*T]   x transposed on host
  qT/kT/vT [128=2h*64, T]  per batch, produced by the QKV matmuls
  S^T   [tk, tq]   scores transposed => softmax row-sum over partitions is a
                   ones-matmul; PV needs exactly this layout
  outT  [C, B*T]   partial output, transposed back on host

Softmax row-sums ride inside the PV matmuls: the stationary operand is
[v_h | pad_bcast], so PSUM rows 0:64 accumulate Y_h and rows 64:128 the
pad-masked row-sum (replicated). A constant "swap" matmul mirrors the two
row-sum halves onto the opposite partition halves so the normalize multiply
is partition-aligned.

All matmuls run as float32r (fp32 read, fp22 multiply) which is full
TensorE rate (1 cycle/row) at free-dim >= 256, ~4x faster than true fp32.
fp32r matmuls cannot target PSUM partition offsets (col tile_position != 0
fails walrus codegen), which is why no column packing is used.
"""

import numpy as np
from contextlib import ExitStack

import concourse.bass as bass
import concourse.bacc as bacc
import concourse.mybir as mybir
import concourse.tile as tile
from concourse import bass_utils
from concourse.masks import make_identity

B, T, C = 4, 2048, 1024
H, D = 16, 64
NCORES = 8
HPC = H // NCORES            # heads per core = 2
CPC = HPC * D                # y-channels per core = 128
BT = B * T                   # 8192
F = 3 * CPC                  # qkv rows per core = 384
TQB = 512                    # tq block (matmul free dim)
NJ = T // TQB                # 4 tq blocks per batch
NKT = T // 128               # 16 tk tiles per batch
NCT = C // 128               # 8 contraction tiles for projections
FP32 = mybir.dt.float32
FP32R = mybir.dt.float32r
AF = mybir.ActivationFunctionType
SCALE = 1.0 / np.sqrt(D)

_cached = {}

# build-time config knobs (A/B testing)
CFG = {
    "batched_dma": False,    # per-c-tile DMAs beat one 3D DMA on HW
    "mask_engine": "vector", # DVE mask beats gpsimd (pool 2-input is slow)
    "spsum_bufs": 2,
    "accps_bufs": 1,
    "exp_mode": "combined",     # "split": per-head [128,512] exp; "combined": one [128,1024] exp

    "qkps_bufs": 2,
    "ppool_bufs": 8,
    "bias_engine": "scalar",  # "scalar": ACT Identity+bias; "vector": DVE add
}


def _emit(tc, nc, xT, wqkvT, bqkv, woutT, padT, outT, reps=1):
    ctx = ExitStack()
    with ctx:
        const = ctx.enter_context(tc.tile_pool(name="const", bufs=1))
        xpool = ctx.enter_context(tc.tile_pool(name="xpool", bufs=2))
        qkvpool = ctx.enter_context(tc.tile_pool(name="qkvpool", bufs=2))
        ppool = ctx.enter_context(tc.tile_pool(name="ppool", bufs=CFG["ppool_bufs"]))
        ypool = ctx.enter_context(tc.tile_pool(name="ypool", bufs=2))
        opool = ctx.enter_context(tc.tile_pool(name="opool", bufs=3))
        spsum = ctx.enter_context(tc.tile_pool(name="spsum", bufs=CFG["spsum_bufs"], space="PSUM"))
        accps = ctx.enter_context(tc.tile_pool(name="accps", bufs=CFG["accps_bufs"], space="PSUM"))
        qkps = ctx.enter_context(tc.tile_pool(name="qkps", bufs=CFG["qkps_bufs"], space="PSUM"))

        # ---- constants ----
        identity = const.tile([128, 128], FP32)
        make_identity(nc, identity)
        # swap matrix: mirrors partition halves (and scales by 1/64 to undo
        # the 64-fold replication summed by the swap matmul). Built in fp32,
        # then copied through DVE so the fp32r operand counts as rounded.
        swap_f32 = const.tile([128, 128], FP32)
        nc.vector.memset(swap_f32, 0.0)
        nc.vector.memset(swap_f32[0:64, 64:128], 1.0 / 64.0)
        nc.vector.memset(swap_f32[64:128, 0:64], 1.0 / 64.0)
        swapm = const.tile([128, 128], FP32R)
        nc.vector.tensor_copy(swapm, swap_f32)
        # 4 diagonal-block causal masks, each replicated for the 2 heads:
        # mask2[m][p, h*512 + q] = 1.0 if p <= q - 128*m else 0.0
        mask2 = []
        for m in range(4):
            mk = const.tile([128, 2 * TQB], mybir.dt.bfloat16, name=f"mask2_{m}")
            nc.gpsimd.memset(mk, 1.0)
            for h in range(2):
                nc.gpsimd.affine_select(
                    out=mk[:, h * TQB:(h + 1) * TQB],
                    in_=mk[:, h * TQB:(h + 1) * TQB],
                    compare_op=mybir.AluOpType.is_ge,
                    fill=0.0,
                    base=-128 * m,
                    pattern=[[1, TQB]],
                    channel_multiplier=-1,
                )
            mask2.append(mk)

        # weights
        w_sb = const.tile([128, NCT, F], FP32R)     # wqkvT tiles: [c-tile][f]
        for ct in range(NCT):
            nc.sync.dma_start(w_sb[:, ct, :], wqkvT[ct * 128:(ct + 1) * 128, :])
        b_sb = const.tile([128, 3], FP32)
        for ft in range(3):
            nc.gpsimd.dma_start(b_sb[:, ft:ft + 1],
                                bqkv[ft * 128:(ft + 1) * 128].unsqueeze(1))
        wo_sb = const.tile([128, C], FP32R)         # woutT [cy, o]
        nc.sync.dma_start(wo_sb, woutT)
        bb_sb = None
        if CFG["bias_engine"] == "vector":
            # bias broadcast along free dim, built once on ACT (zero input +
            # per-partition bias), consumed by DVE adds in steady state
            zb = const.tile([128, TQB], FP32)
            nc.vector.memset(zb, 0.0)
            bb_sb = const.tile([128, 3, TQB], FP32)
            for ft in range(3):
                nc.scalar.activation(bb_sb[:, ft, :], zb, AF.Identity,
                                     bias=b_sb[:, ft:ft + 1])

        for rep in range(reps):
            for b in range(B):
                # ---- QKV projection for this batch: qT/kT/vT [128, T] ----
                qkv_sb = qkvpool.tile([128, 3, T], FP32R, name=f"{rep}_qkv_{b}", tag="qkv")
                for jj in range(NJ):
                    tb = b * NJ + jj
                    x_sb = xpool.tile([128, NCT, TQB], FP32R, name=f"{rep}_x_{tb}", tag="x")
                    if CFG["batched_dma"]:
                        nc.sync.dma_start(
                            x_sb,
                            xT[:, tb * TQB:(tb + 1) * TQB].rearrange(
                                "(ct p) q -> p ct q", p=128))
                    else:
                        for ct in range(NCT):
                            nc.sync.dma_start(
                                x_sb[:, ct, :],
                                xT[ct * 128:(ct + 1) * 128,
                                   tb * TQB:(tb + 1) * TQB])
                    for ft in range(3):
                        ps = qkps.tile([128, TQB], FP32, name=f"{rep}_qkvps_{tb}_{ft}",
                                       tag="qk")
                        for ct in range(NCT):
                            nc.tensor.matmul(
                                ps,
                                lhsT=w_sb[:, ct, ft * 128:(ft + 1) * 128],
                                rhs=x_sb[:, ct, :],
                                start=(ct == 0), stop=(ct == NCT - 1))
                        # bias-add + copy to SBUF (Identity is resident in
                        # every ACT table set, incl. exp's)
                        if CFG["bias_engine"] == "vector":
                            nc.vector.tensor_add(
                                qkv_sb[:, ft, jj * TQB:(jj + 1) * TQB], ps,
                                bb_sb[:, ft, :])
                        else:
                            nc.scalar.activation(
                                qkv_sb[:, ft, jj * TQB:(jj + 1) * TQB], ps,
                                AF.Identity, bias=b_sb[:, ft:ft + 1])
                q_sb = qkv_sb[:, 0, :]
                k_sb = qkv_sb[:, 1, :]
                vT_sb = qkv_sb[:, 2, :]

                # pad value replicated along the free dim (host pre-broadcast):
                # pbc[p, i, f] = pad[b, i*128 + p]
                pbc_sb = qkvpool.tile([128, NKT, 128], FP32R, name=f"{rep}_pbc_{b}",
                                      tag="pbc")
                nc.sync.dma_start(
                    pbc_sb, padT[b].rearrange("(i p) f -> p i f", p=128))

                # ---- transpose V to [tk, d]; build augmented PV stationaries
                #      vA = [v_h0 * pad | pad], vB = [pad | v_h1 * pad]
                # The pad halves are DMAed straight from DRAM (no DVE copies).
                vA_sb = qkvpool.tile([128, NKT, 128], FP32R, name=f"{rep}_vA_{b}",
                                     tag="vA")
                vB_sb = qkvpool.tile([128, NKT, 128], FP32R, name=f"{rep}_vB_{b}",
                                     tag="vB")
                nc.sync.dma_start(
                    vA_sb[:, :, 64:128],
                    padT[b, :, 0:64].rearrange("(i p) f -> p i f", p=128))
                nc.sync.dma_start(
                    vB_sb[:, :, 0:64],
                    padT[b, :, 0:64].rearrange("(i p) f -> p i f", p=128))
                for i in range(NKT):
                    pvt = qkps.tile([128, 128], FP32, name=f"{rep}_vt_{b}_{i}", tag="qk")
                    nc.tensor.transpose(pvt,
                                        vT_sb[:, i * 128:(i + 1) * 128].bitcast(
                                            FP32),
                                        identity)
                    nc.vector.tensor_mul(vA_sb[:, i, 0:64], pvt[:, 0:64],
                                         pbc_sb[:, i, 0:64])
                    nc.vector.tensor_mul(vB_sb[:, i, 64:128], pvt[:, 64:128],
                                         pbc_sb[:, i, 64:128])

                # ---- attention per tq block ----
                for j in range(NJ):
                    ntk = 4 * (j + 1)
                    pyA = accps.tile([128, TQB], FP32, name=f"{rep}_pyA_{b}_{j}",
                                     tag="pyA")
                    pyB = accps.tile([128, TQB], FP32, name=f"{rep}_pyB_{b}_{j}",
                                     tag="pyB")
                    for i in range(ntk):
                        p_sb = ppool.tile([128, 2 * TQB], FP32R,
                                          name=f"{rep}_p_{b}_{j}_{i}", tag="p")
                        if CFG["exp_mode"] == "combined":
                            ps2 = spsum.tile([128, 2 * TQB], FP32,
                                             name=f"{rep}_s_{b}_{j}_{i}",
                                             tag="s")
                            for h in range(2):
                                nc.tensor.matmul(
                                    ps2[:, h * TQB:(h + 1) * TQB],
                                    lhsT=k_sb[h * 64:(h + 1) * 64,
                                              i * 128:(i + 1) * 128],
                                    rhs=q_sb[h * 64:(h + 1) * 64,
                                             j * TQB:(j + 1) * TQB],
                                    start=True, stop=True,
                                    tile_position=(h * 64, 0))
                            nc.scalar.activation(p_sb, ps2, AF.Exp,
                                                 scale=float(SCALE))
                        else:
                            for h in range(2):
                                ps = spsum.tile([128, TQB], FP32,
                                                name=f"{rep}_s_{b}_{j}_{i}_{h}",
                                                tag="s")
                                nc.tensor.matmul(
                                    ps,
                                    lhsT=k_sb[h * 64:(h + 1) * 64,
                                              i * 128:(i + 1) * 128],
                                    rhs=q_sb[h * 64:(h + 1) * 64,
                                             j * TQB:(j + 1) * TQB],
                                    start=True, stop=True,
                                    tile_position=(h * 64, 0))
                                nc.scalar.activation(
                                    p_sb[:, h * TQB:(h + 1) * TQB], ps,
                                    AF.Exp, scale=float(SCALE))
                        if i >= 4 * j:
                            # gpsimd: DVE is the busiest engine, Pool is idle,
                            # and all three operands are SBUF (Pool can't touch
                            # PSUM)
                            if CFG["mask_engine"] == "pool":
                                nc.gpsimd.tensor_mul(p_sb, p_sb,
                                                     mask2[i - 4 * j])
                            else:
                                nc.vector.tensor_mul(p_sb, p_sb,
                                                     mask2[i - 4 * j])
                        first, last = (i == 0), (i == ntk - 1)
                        # rows 0:64 <- Y_h0, rows 64:128 <- rowsum_h0 (x64)
                        nc.tensor.matmul(pyA, lhsT=vA_sb[:, i, :],
                                         rhs=p_sb[:, 0:TQB],
                                         start=first, stop=last)
                        # rows 0:64 <- rowsum_h1 (x64), rows 64:128 <- Y_h1
                        nc.tensor.matmul(pyB, lhsT=vB_sb[:, i, :],
                                         rhs=p_sb[:, TQB:2 * TQB],
                                         start=first, stop=last)

                    # assemble [rowsum_h1 | rowsum_h0] and mirror the halves so
                    # each Y row sees its own head's row-sum
                    rs_sb = ypool.tile([128, TQB], FP32R, name=f"{rep}_rs_{b}_{j}",
                                       tag="rs")
                    nc.vector.tensor_copy(rs_sb[0:64, :], pyB[0:64, :])
                    nc.vector.tensor_copy(rs_sb[64:128, :], pyA[64:128, :])
                    prs = qkps.tile([128, TQB], FP32, name=f"{rep}_prs_{b}_{j}",
                                    tag="qk")
                    nc.tensor.matmul(prs, lhsT=swapm, rhs=rs_sb, start=True,
                                     stop=True)
                    recip = ypool.tile([128, TQB], FP32, name=f"{rep}_rc_{b}_{j}",
                                       tag="rc")
                    nc.vector.reciprocal(recip, prs)
                    y_sb = ypool.tile([128, TQB], FP32R, name=f"{rep}_y_{b}_{j}",
                                      tag="y")
                    nc.vector.tensor_mul(y_sb[0:64, :], pyA[0:64, :],
                                         recip[0:64, :])
                    nc.vector.tensor_mul(y_sb[64:128, :], pyB[64:128, :],
                                         recip[64:128, :])

                    # ---- out projection for this tq block ----
                    if CFG["batched_dma"]:
                        for og in range(2):
                            o_sb = opool.tile([128, NCT // 2, TQB], FP32,
                                              name=f"{rep}_o_{b}_{j}_{og}",
                                              tag="o")
                            for oi in range(NCT // 2):
                                ot = og * (NCT // 2) + oi
                                po = qkps.tile([128, TQB], FP32,
                                               name=f"{rep}_po_{b}_{j}_{ot}",
                                               tag="qk")
                                nc.tensor.matmul(
                                    po,
                                    lhsT=wo_sb[:, ot * 128:(ot + 1) * 128],
                                    rhs=y_sb, start=True, stop=True)
                                nc.vector.tensor_copy(o_sb[:, oi, :], po)
                            nc.sync.dma_start(
                                outT[og * 512:(og + 1) * 512,
                                     b * T + j * TQB:b * T + (j + 1) * TQB]
                                .rearrange("(ot p) q -> p ot q", p=128), o_sb)
                    else:
                        for ot in range(NCT):
                            po = qkps.tile([128, TQB], FP32,
                                           name=f"{rep}_po_{b}_{j}_{ot}",
                                           tag="qk")
                            nc.tensor.matmul(
                                po, lhsT=wo_sb[:, ot * 128:(ot + 1) * 128],
                                rhs=y_sb, start=True, stop=True)
                            o_sb = opool.tile([128, TQB], FP32,
                                              name=f"{rep}_o_{b}_{j}_{ot}",
                                              tag="o")
                            nc.vector.tensor_copy(o_sb, po)
                            nc.sync.dma_start(
                                outT[ot * 128:(ot + 1) * 128,
                                     b * T + j * TQB:b * T + (j + 1) * TQB],
                                o_sb)


def build(reps=1):
    nc = bacc.Bacc()
    xT = nc.dram_tensor("xT", [C, BT], FP32R, kind="ExternalInput")
    wqkvT = nc.dram_tensor("wqkvT", [C, F], FP32R, kind="ExternalInput")
    bqkv = nc.dram_tensor("bqkv", [F], FP32, kind="ExternalInput")
    woutT = nc.dram_tensor("woutT", [CPC, C], FP32R, kind="ExternalInput")
    padT = nc.dram_tensor("padT", [B, T, 128], FP32R, kind="ExternalInput")
    outT = nc.dram_tensor("outT", [C, BT], FP32, kind="ExternalOutput")
    with tile.TileContext(nc) as tc:
        _emit(tc, nc, xT.ap(), wqkvT.ap(), bqkv.ap(), woutT.ap(), padT.ap(),
              outT.ap(), reps=reps)
    nc.compile()
    return nc


def make_in_maps(x, attention_mask, Wqkv, bqkv, Wout):
    xT = np.ascontiguousarray(
        x.reshape(BT, C).T.astype(np.float32, copy=False))
    padT = np.ascontiguousarray(np.broadcast_to(
        attention_mask.astype(np.float32)[:, :, None], (B, T, 128)))
    in_maps = []
    for c in range(NCORES):
        rows = np.r_[c * CPC:(c + 1) * CPC,
                     C + c * CPC:C + (c + 1) * CPC,
                     2 * C + c * CPC:2 * C + (c + 1) * CPC]
        wqkvT_c = np.ascontiguousarray(Wqkv[rows, :].T.astype(np.float32,
                                                              copy=False))
        b_c = np.ascontiguousarray(bqkv[rows].astype(np.float32, copy=False))
        woutT_c = np.ascontiguousarray(
            Wout[:, c * CPC:(c + 1) * CPC].T.astype(np.float32, copy=False))
        in_maps.append({"xT": xT, "wqkvT": wqkvT_c, "bqkv": b_c,
                        "woutT": woutT_c, "padT": padT})
    return in_maps


def kernel(x, attention_mask, Wqkv, bqkv, Wout, _trace=False):
    x = np.asarray(x)
    attention_mask = np.asarray(attention_mask)
    Wqkv = np.asarray(Wqkv)
    bqkv = np.asarray(bqkv)
    Wout = np.asarray(Wout)
    if "nc" not in _cached:
        _cached["nc"] = build()
    nc = _cached["nc"]
    in_maps = make_in_maps(x, attention_mask, Wqkv, bqkv, Wout)
    res = bass_utils.run_bass_kernel_spmd(
        nc, in_maps, core_ids=list(range(NCORES)), trace=_trace)
    acc = res.results[0]["outT"].astype(np.float32)
    for r in res.results[1:]:
        acc += r["outT"]
    out = np.ascontiguousarray(acc.T).reshape(B, T, C).astype(np.float32)
    if _trace:
        _cached["last_result"] = res
    return out



# revision 3
# speedup vs baseline: 52.1325x; 52.1325x over previous
"""Causal self-attention (B=4, T=2048, C=1024, H=16) on 8 trn2 NeuronCores.

Sharding v2: batch x head-group. Core c handles batch b=c//2 and head group
g2=c%2 (8 heads = 4 head-pairs). Each core:
 - QKV projection for its 8 heads over its batch's 2048 tokens
 - attention for 4 head-pairs (causal, diagonal-sub-sliced)
 - out-projection partial [C, T] contracted over its 512 y-channels
Host sums the two partials per batch (the "all-reduce"), 8.4 MB each
(vs 33.5 MB x 8 in the pure head-parallel variant -> 3.4x less HBM traffic).

Layouts per core (partition dim first everywhere):
  xT    [C, T]           x[b] transposed on host, fp32r
  q/k   [128=2h*64, hp, T]  bf16 (q transient per tq block, k persistent)
  vAB   [128 tk, hp, 16, 3, 64] bf16: [v_h0 | pad/64 | v_h1] per tk tile
  S^T   [tk, 2, tq]      scores transposed; exp -> p bf16
  pyA = vA.T@p_h0 = [Y_h0 | rs_h0], pyB = vB.T@p_h1 = [rs_h1 | Y_h1]
  swap matmuls (bf16) mirror the replicated row-sum halves so the
  normalize multiply is partition-aligned.
  outT  [C, T] bf16 partial, summed+transposed on host.

Matmul dtypes: projections fp32r (fp22 multiply, 1 cyc/row at N>=256);
attention bf16 (1 cyc/row at any N, halves SBUF + 4x DVE mask ops).
Diagonal tiles only compute the causally-live column range (N=512-128*di).
"""

import numpy as np
import ml_dtypes
from contextlib import ExitStack

import concourse.bass as bass
import concourse.bacc as bacc
import concourse.mybir as mybir
import concourse.tile as tile
from concourse import bass_utils
from concourse.masks import make_identity

B, T, C = 4, 2048, 1024
H, D = 16, 64
NCORES = 8
G = 4                 # head-pairs per core
NCT = C // 128        # 8 contraction tiles for projections
TQB = 512             # tq block
NJ = T // TQB         # 4
NKT = T // 128        # 16
FP32 = mybir.dt.float32
FP32R = mybir.dt.float32r
BF16 = mybir.dt.bfloat16
AF = mybir.ActivationFunctionType
SCALE = 1.0 / np.sqrt(D)

_cached = {}

CFG = {
    "ppool_bufs": 8,
    "spsum_bufs": 2,
    "qkps_bufs": 2,
    "ypool_bufs": 16,
    "drainA_engine": "vector",   # pyA -> ya drain
    "drainB_engine": "vector",   # pyB -> yb drain
}


def _emit(tc, nc, xT, wq, bq, wo, padb, padk, outT, reps=1,
          ones_mask=False):
    ctx = ExitStack()
    with ctx:
        const = ctx.enter_context(tc.tile_pool(name="const", bufs=1))
        xpool = ctx.enter_context(tc.tile_pool(name="xpool", bufs=2))
        qpool = ctx.enter_context(tc.tile_pool(name="qpool", bufs=2))
        vpool = ctx.enter_context(tc.tile_pool(name="vpool", bufs=5))
        ppool = ctx.enter_context(tc.tile_pool(name="ppool", bufs=CFG["ppool_bufs"]))
        yab = ctx.enter_context(tc.tile_pool(name="yab", bufs=2))
        rcpool = ctx.enter_context(tc.tile_pool(name="rcpool", bufs=2))
        ypool = ctx.enter_context(tc.tile_pool(name="ypool", bufs=CFG["ypool_bufs"]))
        opool = ctx.enter_context(tc.tile_pool(name="opool", bufs=2))
        spsum = ctx.enter_context(
            tc.tile_pool(name="spsum", bufs=CFG["spsum_bufs"], space="PSUM"))
        accps = ctx.enter_context(tc.tile_pool(name="accps", bufs=1, space="PSUM"))
        qkps = ctx.enter_context(
            tc.tile_pool(name="qkps", bufs=CFG["qkps_bufs"], space="PSUM"))

        # ---- constants ----
        # weights. w block DMAs are deferred into the first A(j) pass so the
        # first projection matmuls only wait on their own block, not the full
        # 6.3 MB load; DMAs rotate across engine queues for parallelism.
        dma_engines = [nc.sync, nc.scalar, nc.gpsimd]
        b_sb = const.tile([128, 3 * G], FP32)
        nc.gpsimd.dma_start(b_sb, bq)
        # weights: coarse per-ct DMAs on the scalar/gpsimd queues; the sync
        # queue is left free so A(0)'s x tiles land immediately.
        w_sb = const.tile([128, NCT, 3 * G * 128], BF16)
        for ct in range(NCT):
            (nc.scalar if ct % 2 == 0 else nc.gpsimd).dma_start(
                w_sb[:, ct, :], wq[ct * 128:(ct + 1) * 128, :])
        # k and vAB persist across the rep (rewritten per rep)
        k_sb = const.tile([128, G, T], BF16)
        vAB = const.tile([128, G, NKT, 3, 64], BF16)
        if ones_mask:
            padb_sb = None
            nc.gpsimd.memset(vAB[:, :, :, 1, :], 1.0)
        else:
            padb_sb = const.tile([128, T], BF16)
            nc.sync.dma_start(padb_sb, padb)
            for hp in range(G):
                nc.scalar.dma_start(vAB[:, hp, :, 1, :],
                                    padk.rearrange("p (i f) -> p i f", f=64))
        wo_sb = const.tile([128, G, C], FP32R)

        # gpsimd const builds go after the DMA kickoffs so they don't delay
        # the gpsimd DMA queue at startup
        ident = const.tile([128, 128], BF16)
        make_identity(nc, ident)
        tri = const.tile([128, 2, 128], BF16)
        nc.gpsimd.memset(tri, 1.0)
        for h in range(2):
            nc.gpsimd.affine_select(
                out=tri[:, h, :], in_=tri[:, h, :],
                compare_op=mybir.AluOpType.is_ge, fill=0.0,
                base=0, pattern=[[1, 128]], channel_multiplier=-1)

        def load_wo():
            for hp in range(G):
                for ot in range(0, NCT, 2):
                    dma_engines[(hp + ot) % len(dma_engines)].dma_start(
                        wo_sb[:, hp, ot * 128:(ot + 2) * 128],
                        wo[hp * 128:(hp + 1) * 128,
                           ot * 128:(ot + 2) * 128])

        for rep in range(reps):
            q_tiles = {}
            y_tiles = {}

            def emit_A(j):
                x = xpool.tile([128, NCT, TQB], BF16, name=f"{rep}_x_{j}",
                               tag="x")
                for ct in range(NCT):
                    nc.sync.dma_start(
                        x[:, ct, :],
                        xT[ct * 128:(ct + 1) * 128, j * TQB:(j + 1) * TQB])
                q = qpool.tile([128, G, TQB], BF16, name=f"{rep}_q_{j}", tag="q")
                q_tiles[j] = q
                v_tiles = {}
                for hp in range(G):
                    for ft in range(3):
                        ps = qkps.tile([128, TQB], FP32,
                                       name=f"{rep}_ps_{j}_{hp}_{ft}", tag="ps")
                        for ct in range(NCT):
                            nc.tensor.matmul(
                                ps,
                                lhsT=w_sb[:, ct,
                                          (hp * 3 + ft) * 128:
                                          (hp * 3 + ft + 1) * 128],
                                rhs=x[:, ct, :],
                                start=(ct == 0), stop=(ct == NCT - 1))
                        col = hp * 3 + ft
                        # bias-adds on DVE: ACT is the binding engine during
                        # attention (exp), DVE has slack
                        if ft == 0:
                            nc.vector.tensor_scalar(
                                q[:, hp, :], ps, float(SCALE),
                                b_sb[:, col:col + 1],
                                mybir.AluOpType.mult, mybir.AluOpType.add)
                        elif ft == 1:
                            nc.vector.tensor_scalar_add(
                                k_sb[:, hp, j * TQB:(j + 1) * TQB], ps,
                                b_sb[:, col:col + 1])
                        else:
                            v = vpool.tile([128, TQB], BF16,
                                           name=f"{rep}_v_{j}_{hp}", tag="v")
                            nc.vector.tensor_scalar_add(
                                v, ps, b_sb[:, col:col + 1])
                            if not ones_mask:
                                nc.vector.tensor_mul(
                                    v, v, padb_sb[:, j * TQB:(j + 1) * TQB])
                            v_tiles[hp] = v
                # transposes go after all projections: in-order PE would
                # otherwise stall on the ACT/DVE v pipeline mid-phase
                for hp in range(G):
                    v = v_tiles[hp]
                    pb = qkps.tile([128, 4, 2, 64], BF16,
                                   name=f"{rep}_pb_{j}_{hp}", tag="ps")
                    for t in range(4):
                        nc.tensor.transpose(
                            pb[:, t, :, :], v[:, t * 128:(t + 1) * 128], ident)
                    for h in range(2):
                        nc.vector.tensor_copy(
                            vAB[:, hp, j * 4:(j + 1) * 4, 2 * h, :],
                            pb[:, :, h, :])

            def emit_B(j, hp):
                q = q_tiles[j]
                pyA = accps.tile([128, TQB], FP32, name=f"{rep}_pyA_{j}_{hp}",
                                 tag="pyA")
                pyB = accps.tile([128, TQB], FP32, name=f"{rep}_pyB_{j}_{hp}",
                                 tag="pyB")
                ntk = 4 * (j + 1)
                # interleave the 4 diagonal tiles (small-N matmuls but
                # full-size exp -> ACT-heavy) among the full tiles so the
                # ACT engine never falls behind PE in a burst. First element
                # must be a lo=0 tile (PV start=True zeroes the full range).
                fulls = list(range(4 * j))
                diags = [4 * j + d for d in range(4)]
                if j == 0:
                    seq = diags
                else:
                    seq, k0 = [], 0
                    for d in diags:
                        seq += fulls[k0:k0 + j]
                        seq.append(d)
                        k0 += j
                    seq += fulls[k0:]

                def emit_S(i):
                    di = i - 4 * j
                    lo = 128 * di if di > 0 else 0
                    s2 = spsum.tile([128, 2, TQB], FP32,
                                    name=f"{rep}_s_{j}_{hp}_{i}", tag="s")
                    for h in range(2):
                        nc.tensor.matmul(
                            s2[:, h, lo:TQB],
                            lhsT=k_sb[h * 64:(h + 1) * 64, hp,
                                      i * 128:(i + 1) * 128],
                            rhs=q[h * 64:(h + 1) * 64, hp, lo:TQB],
                            start=True, stop=True,
                            tile_position=(h * 64, 0))
                    p = ppool.tile([128, 2, TQB], BF16,
                                   name=f"{rep}_p_{j}_{hp}_{i}", tag="p")
                    nc.scalar.activation(p[:, :, lo:TQB], s2[:, :, lo:TQB],
                                         AF.Exp)
                    if di >= 0:
                        nc.vector.tensor_mul(p[:, :, lo:lo + 128],
                                             p[:, :, lo:lo + 128], tri)
                    return p, lo

                def emit_PV(i, p, lo, first, last):
                    nc.tensor.matmul(pyA[:, lo:TQB],
                                     lhsT=vAB[:, hp, i, 0:2, :],
                                     rhs=p[:, 0, lo:TQB],
                                     start=first, stop=last)
                    nc.tensor.matmul(pyB[:, lo:TQB],
                                     lhsT=vAB[:, hp, i, 1:3, :],
                                     rhs=p[:, 1, lo:TQB],
                                     start=first, stop=last)

                # software skew: S one step ahead of PV so PV never waits exp
                pending = None
                for n, i in enumerate(seq):
                    p, lo = emit_S(i)
                    if pending is not None:
                        emit_PV(*pending, first=(n == 1), last=False)
                    pending = (i, p, lo)
                emit_PV(*pending, first=(ntk == 1), last=True)
                # drain PSUM accumulators (frees banks for the next unit)
                ya = yab.tile([128, TQB], BF16, name=f"{rep}_ya_{j}_{hp}",
                              tag="ya")
                yb = yab.tile([128, TQB], BF16, name=f"{rep}_yb_{j}_{hp}",
                              tag="yb")
                if CFG["drainA_engine"] == "vector":
                    nc.vector.tensor_copy(ya, pyA)
                else:
                    nc.scalar.activation(ya, pyA, AF.Copy)
                if CFG["drainB_engine"] == "scalar":
                    nc.scalar.activation(yb, pyB, AF.Copy)
                else:
                    nc.vector.tensor_copy(yb, pyB)
                # align the row-sums with their Y halves: stream_shuffle
                # with an identity mask and offset APs is a cross-partition
                # copy on DVE (no PE, no PSUM slot)
                rs_al = rcpool.tile([128, TQB], BF16,
                                    name=f"{rep}_rs_{j}_{hp}", tag="rs")
                idmask = list(range(32))
                nc.vector.stream_shuffle(rs_al[0:64, :], ya[64:128, :], idmask)
                nc.vector.stream_shuffle(rs_al[64:128, :], yb[0:64, :], idmask)
                recip = rcpool.tile([128, TQB], FP32, name=f"{rep}_rc_{j}_{hp}",
                                    tag="rc")
                nc.vector.reciprocal(recip, rs_al)
                y = ypool.tile([128, TQB], FP32R, name=f"{rep}_y_{j}_{hp}",
                               tag="y")
                y_tiles[(j, hp)] = y
                nc.vector.tensor_mul(y[0:64, :], ya[0:64, :], recip[0:64, :])
                nc.vector.tensor_mul(y[64:128, :], yb[64:128, :],
                                     recip[64:128, :])

            def emit_C(j, ots=None):
                for ot in (range(NCT) if ots is None else ots):
                    po = qkps.tile([128, TQB], FP32, name=f"{rep}_po_{j}_{ot}",
                                   tag="ps")
                    for hp in range(G):
                        nc.tensor.matmul(
                            po, lhsT=wo_sb[:, hp, ot * 128:(ot + 1) * 128],
                            rhs=y_tiles[(j, hp)],
                            start=(hp == 0), stop=(hp == G - 1))
                    o = opool.tile([128, TQB], BF16, name=f"{rep}_o_{j}_{ot}",
                                   tag="o")
                    if j == NJ - 1 and ot % 2 == 1:
                        nc.scalar.activation(o, po, AF.Copy)
                    else:
                        nc.vector.tensor_copy(o, po)
                    nc.sync.dma_start(
                        outT[ot * 128:(ot + 1) * 128,
                             j * TQB:(j + 1) * TQB], o)

            emit_A(0)
            emit_A(1)
            if rep == 0:
                load_wo()
            for hp in range(G):
                emit_B(0, hp)
            emit_A(2)
            # interleave out-projections (pure PE) between B units as filler
            # for the exp-throughput deficit of the attention stretches
            for hp in range(G):
                emit_B(1, hp)
                emit_C(0, [2 * hp, 2 * hp + 1])
            emit_A(3)
            for hp in range(G):
                emit_B(2, hp)
                emit_C(1, [2 * hp, 2 * hp + 1])
            for hp in range(G):
                emit_B(3, hp)
                emit_C(2, [2 * hp, 2 * hp + 1])
            emit_C(3)


def build(reps=1, ones_mask=False):
    nc = bacc.Bacc()
    xT = nc.dram_tensor("xT", [C, T], BF16, kind="ExternalInput")
    wq = nc.dram_tensor("wq", [C, 3 * G * 128], BF16, kind="ExternalInput")
    bq = nc.dram_tensor("bq", [128, 3 * G], FP32, kind="ExternalInput")
    wo = nc.dram_tensor("wo", [G * 128, C], FP32R, kind="ExternalInput")
    padb = nc.dram_tensor("padb", [128, T], BF16, kind="ExternalInput")
    padk = nc.dram_tensor("padk", [128, NKT * 64], BF16, kind="ExternalInput")
    outT = nc.dram_tensor("outT", [C, T], BF16, kind="ExternalOutput")
    with tile.TileContext(nc) as tc:
        _emit(tc, nc, xT.ap(), wq.ap(), bq.ap(), wo.ap(), padb.ap(),
              padk.ap(), outT.ap(), reps=reps, ones_mask=ones_mask)
    nc.compile()
    return nc


def make_in_maps(x, attention_mask, Wqkv, bqkv, Wout):
    bf = ml_dtypes.bfloat16
    in_maps = []
    xTb, padbb, padkb = [], [], []
    for b in range(B):
        xTb.append(np.ascontiguousarray(x[b].T).astype(bf))
        pad = attention_mask[b].astype(np.float32)
        padbb.append(np.ascontiguousarray(
            np.broadcast_to(pad[None, :], (128, T))).astype(bf))
        pk = np.broadcast_to(
            pad.reshape(16, 128, 1), (16, 128, 64))
        padkb.append(np.ascontiguousarray(
            pk.transpose(1, 0, 2).reshape(128, 16 * 64)).astype(bf))
    for c in range(NCORES):
        b, g2 = c // 2, c % 2
        wq_blocks, bq_blocks, wo_blocks = [], [], []
        for hp in range(G):
            h0 = 8 * g2 + 2 * hp
            rows2 = np.r_[64 * h0:64 * h0 + 128]
            for ft in range(3):
                rows = ft * C + rows2
                wq_blocks.append(Wqkv[rows, :])
                bq_blocks.append(bqkv[rows] * (0.125 if ft == 0 else 1.0))
            wo_blocks.append(Wout[:, rows2].T)
        wq_c = np.ascontiguousarray(
            np.concatenate(wq_blocks, 0).T).astype(bf)
        bq_c = np.ascontiguousarray(
            np.stack(bq_blocks, 1).astype(np.float32, copy=False))
        wo_c = np.ascontiguousarray(
            np.concatenate(wo_blocks, 0).astype(np.float32, copy=False))
        in_maps.append({"xT": xTb[b], "wq": wq_c, "bq": bq_c, "wo": wo_c,
                       "padb": padbb[b], "padk": padkb[b]})
    return in_maps


def kernel(x, attention_mask, Wqkv, bqkv, Wout, _trace=False):
    x = np.asarray(x)
    attention_mask = np.asarray(attention_mask)
    Wqkv = np.asarray(Wqkv)
    bqkv = np.asarray(bqkv)
    Wout = np.asarray(Wout)
    ones = bool(np.all(attention_mask == 1))
    key = ("nc_ones" if ones else "nc")
    if key not in _cached:
        _cached[key] = build(ones_mask=ones)
    nc = _cached[key]
    in_maps = make_in_maps(x, attention_mask, Wqkv, bqkv, Wout)
    res = bass_utils.run_bass_kernel_spmd(
        nc, in_maps, core_ids=list(range(NCORES)), trace=_trace)
    out = np.empty((B, T, C), np.float32)
    for b in range(B):
        acc = res.results[2 * b]["outT"].astype(np.float32)
        acc += res.results[2 * b + 1]["outT"].astype(np.float32)
        out[b] = acc.T
    if _trace:
        _cached["last_result"] = res
    return out


# revision 5
# speedup vs baseline: 97.9202x; 1.8783x over previous
"""Causal self-attention (B=4, T=2048, C=1024, H=16) on 8 trn2 NeuronCores.

Sharding v2: batch x head-group. Core c handles batch b=c//2 and head group
g2=c%2 (8 heads = 4 head-pairs). Each core:
 - QKV projection for its 8 heads over its batch's 2048 tokens
 - attention for 4 head-pairs (causal, diagonal-sub-sliced)
 - out-projection partial [C, T] contracted over its 512 y-channels
Host sums the two partials per batch (the "all-reduce"), 8.4 MB each
(vs 33.5 MB x 8 in the pure head-parallel variant -> 3.4x less HBM traffic).

Layouts per core (partition dim first everywhere):
  xT    [C, T]           x[b] transposed on host, fp32r
  q/k   [128=2h*64, hp, T]  bf16 (q transient per tq block, k persistent)
  vAB   [128 tk, hp, 16, 3, 64] bf16: [v_h0 | pad/64 | v_h1] per tk tile
  S^T   [tk, 2, tq]      scores transposed; exp -> p bf16
  pyA = vA.T@p_h0 = [Y_h0 | rs_h0], pyB = vB.T@p_h1 = [rs_h1 | Y_h1]
  swap matmuls (bf16) mirror the replicated row-sum halves so the
  normalize multiply is partition-aligned.
  outT  [C, T] bf16 partial, summed+transposed on host.

Matmul dtypes: projections fp32r (fp22 multiply, 1 cyc/row at N>=256);
attention bf16 (1 cyc/row at any N, halves SBUF + 4x DVE mask ops).
Diagonal tiles only compute the causally-live column range (N=512-128*di).
"""

import numpy as np
import ml_dtypes
from contextlib import ExitStack

import concourse.bass as bass
import concourse.bacc as bacc
import concourse.mybir as mybir
import concourse.tile as tile
from concourse import bass_utils
from concourse.masks import make_identity

B, T, C = 4, 2048, 1024
H, D = 16, 64
NCORES = 8
G = 4                 # head-pairs per core
NCT = C // 128        # 8 contraction tiles for projections
TQB = 512             # tq block
NJ = T // TQB         # 4
NKT = T // 128        # 16
FP32 = mybir.dt.float32
FP32R = mybir.dt.float32r
BF16 = mybir.dt.bfloat16
AF = mybir.ActivationFunctionType
SCALE = 1.0 / np.sqrt(D)

_cached = {}

CFG = {
    "ppool_bufs": 12,
    "spsum_bufs": 2,
    "qkps_bufs": 2,
    "ypool_bufs": 20,
    "mask_engine": "vector",
    "yab_bufs": 4,
    "rc_bufs": 4,
    "o_bufs": 4,
    "drainA_engine": "vector",   # pyA -> ya drain
    "drainB_engine": "vector",   # pyB -> yb drain
}


def _emit(tc, nc, xT, wq, bq, wo, padb, padk, outT, reps=1,
          ones_mask=False):
    ctx = ExitStack()
    with ctx:
        const = ctx.enter_context(tc.tile_pool(name="const", bufs=1))
        xpool = ctx.enter_context(tc.tile_pool(name="xpool", bufs=CFG.get("x_bufs", 2)))
        qpool = ctx.enter_context(tc.tile_pool(name="qpool", bufs=CFG.get("q_bufs", 2)))
        vpool = ctx.enter_context(tc.tile_pool(name="vpool", bufs=5))
        ppool = ctx.enter_context(tc.tile_pool(name="ppool", bufs=CFG["ppool_bufs"]))
        yab = ctx.enter_context(tc.tile_pool(name="yab", bufs=CFG.get("yab_bufs", 2)))
        rcpool = ctx.enter_context(tc.tile_pool(name="rcpool", bufs=CFG.get("rc_bufs", 2)))
        ypool = ctx.enter_context(tc.tile_pool(name="ypool", bufs=CFG["ypool_bufs"]))
        opool = ctx.enter_context(tc.tile_pool(name="opool", bufs=CFG.get("o_bufs", 2)))
        spsum = ctx.enter_context(
            tc.tile_pool(name="spsum", bufs=CFG["spsum_bufs"], space="PSUM"))
        accps = ctx.enter_context(tc.tile_pool(name="accps", bufs=1, space="PSUM"))
        qkps = ctx.enter_context(
            tc.tile_pool(name="qkps", bufs=CFG["qkps_bufs"], space="PSUM"))

        # ---- constants ----
        # weights. w block DMAs are deferred into the first A(j) pass so the
        # first projection matmuls only wait on their own block, not the full
        # 6.3 MB load; DMAs rotate across engine queues for parallelism.
        dma_engines = [nc.sync, nc.scalar, nc.gpsimd]
        b_sb = const.tile([128, 3 * G], FP32)
        nc.gpsimd.dma_start(b_sb, bq)
        # weights: coarse per-ct DMAs on the scalar/gpsimd queues; the sync
        # queue is left free so A(0)'s x tiles land immediately.
        w_sb = const.tile([128, NCT, 3 * G * 128], BF16)
        for ct in range(NCT):
            (nc.scalar if ct % 2 == 0 else nc.gpsimd).dma_start(
                w_sb[:, ct, :], wq[ct * 128:(ct + 1) * 128, :])
        # k and vAB persist across the rep (rewritten per rep)
        k_sb = const.tile([128, G, T], BF16)
        vAB = const.tile([128, G, NKT, 3, 64], BF16)
        if ones_mask:
            padb_sb = None
            nc.gpsimd.memset(vAB[:, :, :, 1, :], 1.0)
        else:
            padb_sb = const.tile([128, T], BF16)
            nc.sync.dma_start(padb_sb, padb)
            for hp in range(G):
                nc.scalar.dma_start(vAB[:, hp, :, 1, :],
                                    padk.rearrange("p (i f) -> p i f", f=64))
        wo_sb = const.tile([128, G, C], FP32R)

        # gpsimd const builds go after the DMA kickoffs so they don't delay
        # the gpsimd DMA queue at startup
        ident = const.tile([128, 128], BF16)
        make_identity(nc, ident)
        tri = const.tile([128, 2, 128], BF16)
        nc.gpsimd.memset(tri, 1.0)
        for h in range(2):
            nc.gpsimd.affine_select(
                out=tri[:, h, :], in_=tri[:, h, :],
                compare_op=mybir.AluOpType.is_ge, fill=0.0,
                base=0, pattern=[[1, 128]], channel_multiplier=-1)

        def load_wo():
            for hp in range(G):
                for ot in range(0, NCT, 2):
                    dma_engines[(hp + ot) % len(dma_engines)].dma_start(
                        wo_sb[:, hp, ot * 128:(ot + 2) * 128],
                        wo[hp * 128:(hp + 1) * 128,
                           ot * 128:(ot + 2) * 128])

        pending_c3 = [None]

        for rep in range(reps):
            q_tiles = {}
            y_tiles = {}

            def emit_A(j):
                x = xpool.tile([128, NCT, TQB], BF16, name=f"{rep}_x_{j}",
                               tag="x")
                for ct in range(NCT):
                    nc.sync.dma_start(
                        x[:, ct, :],
                        xT[ct * 128:(ct + 1) * 128, j * TQB:(j + 1) * TQB])
                q = qpool.tile([128, G, TQB], BF16, name=f"{rep}_q_{j}", tag="q")
                q_tiles[j] = q
                v_tiles = {}
                for hp in range(G):
                    for ft in range(3):
                        ps = qkps.tile([128, TQB], FP32,
                                       name=f"{rep}_ps_{j}_{hp}_{ft}", tag="ps")
                        for ct in range(NCT):
                            nc.tensor.matmul(
                                ps,
                                lhsT=w_sb[:, ct,
                                          (hp * 3 + ft) * 128:
                                          (hp * 3 + ft + 1) * 128],
                                rhs=x[:, ct, :],
                                start=(ct == 0), stop=(ct == NCT - 1))
                        col = hp * 3 + ft
                        # bias-adds on DVE: ACT is the binding engine during
                        # attention (exp), DVE has slack
                        if ft == 0:
                            nc.scalar.activation(q[:, hp, :], ps, AF.Identity,
                                                 scale=float(SCALE),
                                                 bias=b_sb[:, col:col + 1])
                        elif ft == 1:
                            nc.scalar.activation(
                                k_sb[:, hp, j * TQB:(j + 1) * TQB], ps,
                                AF.Identity, bias=b_sb[:, col:col + 1])
                        else:
                            v = vpool.tile([128, TQB], BF16,
                                           name=f"{rep}_v_{j}_{hp}", tag="v")
                            nc.vector.tensor_scalar_add(
                                v, ps, b_sb[:, col:col + 1])
                            if not ones_mask:
                                nc.vector.tensor_mul(
                                    v, v, padb_sb[:, j * TQB:(j + 1) * TQB])
                            v_tiles[hp] = v
                # transposes go after all projections: in-order PE would
                # otherwise stall on the ACT/DVE v pipeline mid-phase
                for hp in range(G):
                    v = v_tiles[hp]
                    pb = qkps.tile([128, 4, 2, 64], BF16,
                                   name=f"{rep}_pb_{j}_{hp}", tag="ps")
                    for t in range(4):
                        nc.tensor.transpose(
                            pb[:, t, :, :], v[:, t * 128:(t + 1) * 128], ident)
                    for h in range(2):
                        nc.vector.tensor_copy(
                            vAB[:, hp, j * 4:(j + 1) * 4, 2 * h, :],
                            pb[:, :, h, :])

            def emit_B(j, hp):
                q = q_tiles[j]
                pyA = accps.tile([128, TQB], FP32, name=f"{rep}_pyA_{j}_{hp}",
                                 tag="pyA")
                pyB = accps.tile([128, TQB], FP32, name=f"{rep}_pyB_{j}_{hp}",
                                 tag="pyB")
                ntk = 4 * (j + 1)
                # interleave the 4 diagonal tiles (small-N matmuls but
                # full-size exp -> ACT-heavy) among the full tiles so the
                # ACT engine never falls behind PE in a burst. First element
                # must be a lo=0 tile (PV start=True zeroes the full range).
                fulls = list(range(4 * j))
                diags = [4 * j + d for d in range(4)]
                if j == 0:
                    seq = diags
                else:
                    seq, k0 = [], 0
                    for d in diags:
                        seq += fulls[k0:k0 + j]
                        seq.append(d)
                        k0 += j
                    seq += fulls[k0:]

                def emit_S(i):
                    di = i - 4 * j
                    lo = 128 * di if di > 0 else 0
                    s2 = spsum.tile([128, 2, TQB], FP32,
                                    name=f"{rep}_s_{j}_{hp}_{i}", tag="s")
                    for h in range(2):
                        nc.tensor.matmul(
                            s2[:, h, lo:TQB],
                            lhsT=k_sb[h * 64:(h + 1) * 64, hp,
                                      i * 128:(i + 1) * 128],
                            rhs=q[h * 64:(h + 1) * 64, hp, lo:TQB],
                            start=True, stop=True,
                            tile_position=(h * 64, 0))
                    p = ppool.tile([128, 2, TQB], BF16,
                                   name=f"{rep}_p_{j}_{hp}_{i}", tag="p")
                    nc.scalar.activation(p[:, :, lo:TQB], s2[:, :, lo:TQB],
                                         AF.Exp)
                    if di >= 0:
                        eng = (nc.gpsimd if CFG["mask_engine"] == "pool"
                               else nc.vector)
                        eng.tensor_mul(p[:, :, lo:lo + 128],
                                       p[:, :, lo:lo + 128], tri)
                    return p, lo

                def emit_PV(i, p, lo, first, last):
                    nc.tensor.matmul(pyA[:, lo:TQB],
                                     lhsT=vAB[:, hp, i, 0:2, :],
                                     rhs=p[:, 0, lo:TQB],
                                     start=first, stop=last)
                    nc.tensor.matmul(pyB[:, lo:TQB],
                                     lhsT=vAB[:, hp, i, 1:3, :],
                                     rhs=p[:, 1, lo:TQB],
                                     start=first, stop=last)

                # software skew: S one step ahead of PV so PV never waits exp
                pending = None
                for n, i in enumerate(seq):
                    p, lo = emit_S(i)
                    if pending is not None:
                        emit_PV(*pending, first=(n == 1), last=False)
                    pending = (i, p, lo)
                emit_PV(*pending, first=(ntk == 1), last=True)
                # drain PSUM accumulators (frees banks for the next unit)
                ya = yab.tile([128, TQB], BF16, name=f"{rep}_ya_{j}_{hp}",
                              tag="ya")
                yb = yab.tile([128, TQB], BF16, name=f"{rep}_yb_{j}_{hp}",
                              tag="yb")
                if CFG["drainA_engine"] == "vector":
                    nc.vector.tensor_copy(ya, pyA)
                else:
                    nc.scalar.activation(ya, pyA, AF.Copy)
                if CFG["drainB_engine"] == "scalar":
                    nc.scalar.activation(yb, pyB, AF.Copy)
                else:
                    nc.vector.tensor_copy(yb, pyB)
                # align the row-sums with their Y halves: stream_shuffle
                # with an identity mask and offset APs is a cross-partition
                # copy on DVE (no PE, no PSUM slot)
                rs_al = rcpool.tile([128, TQB], BF16,
                                    name=f"{rep}_rs_{j}_{hp}", tag="rs")
                idmask = list(range(32))
                nc.vector.stream_shuffle(rs_al[0:64, :], ya[64:128, :], idmask)
                nc.vector.stream_shuffle(rs_al[64:128, :], yb[0:64, :], idmask)
                recip = rcpool.tile([128, TQB], FP32, name=f"{rep}_rc_{j}_{hp}",
                                    tag="rc")
                nc.vector.reciprocal(recip, rs_al)
                y = ypool.tile([128, TQB], FP32R, name=f"{rep}_y_{j}_{hp}",
                               tag="y")
                y_tiles[(j, hp)] = y
                nc.vector.tensor_mul(y[0:64, :], ya[0:64, :], recip[0:64, :])
                nc.vector.tensor_mul(y[64:128, :], yb[64:128, :],
                                     recip[64:128, :])

            def emit_C(j, ots=None, yt=None):
                if yt is None:
                    yt = y_tiles
                for ot in (range(NCT) if ots is None else ots):
                    po = qkps.tile([128, TQB], FP32, name=f"{rep}_po_{j}_{ot}",
                                   tag="ps")
                    for hp in range(G):
                        nc.tensor.matmul(
                            po, lhsT=wo_sb[:, hp, ot * 128:(ot + 1) * 128],
                            rhs=yt[(j, hp)],
                            start=(hp == 0), stop=(hp == G - 1))
                    o = opool.tile([128, TQB], BF16, name=f"{rep}_o_{j}_{ot}",
                                   tag="o")
                    if j == NJ - 1 and ot % 2 == 1:
                        nc.scalar.activation(o, po, AF.Copy)
                    else:
                        nc.vector.tensor_copy(o, po)
                    nc.sync.dma_start(
                        outT[ot * 128:(ot + 1) * 128,
                             j * TQB:(j + 1) * TQB], o)

            emit_A(0)
            if pending_c3[0] is not None:
                pending_c3[0]()
                pending_c3[0] = None
            emit_A(1)
            if rep == 0:
                load_wo()
            for hp in range(G):
                emit_B(0, hp)
            emit_A(2)
            # interleave out-projections (pure PE) between B units as filler
            # for the exp-throughput deficit of the attention stretches
            for hp in range(G):
                emit_B(1, hp)
                emit_C(0, [2 * hp, 2 * hp + 1])
            emit_A(3)
            for hp in range(G):
                emit_B(2, hp)
                emit_C(1, [2 * hp, 2 * hp + 1])
            for hp in range(G):
                emit_B(3, hp)
                emit_C(2, [2 * hp, 2 * hp + 1])
            # the last block's out-projection is deferred into the next rep's
            # A(0) window (pure-PE filler there); emitted directly on the
            # final rep
            if rep == reps - 1:
                emit_C(3)
            else:
                pending_c3[0] = (
                    lambda fn=emit_C, yt=y_tiles: fn(3, None, yt))



def build(reps=1, ones_mask=False):
    nc = bacc.Bacc()
    xT = nc.dram_tensor("xT", [C, T], BF16, kind="ExternalInput")
    wq = nc.dram_tensor("wq", [C, 3 * G * 128], BF16, kind="ExternalInput")
    bq = nc.dram_tensor("bq", [128, 3 * G], FP32, kind="ExternalInput")
    wo = nc.dram_tensor("wo", [G * 128, C], FP32R, kind="ExternalInput")
    padb = nc.dram_tensor("padb", [128, T], BF16, kind="ExternalInput")
    padk = nc.dram_tensor("padk", [128, NKT * 64], BF16, kind="ExternalInput")
    outT = nc.dram_tensor("outT", [C, T], BF16, kind="ExternalOutput")
    with tile.TileContext(nc) as tc:
        _emit(tc, nc, xT.ap(), wq.ap(), bq.ap(), wo.ap(), padb.ap(),
              padk.ap(), outT.ap(), reps=reps, ones_mask=ones_mask)
    nc.compile()
    return nc


def make_in_maps(x, attention_mask, Wqkv, bqkv, Wout):
    bf = ml_dtypes.bfloat16
    in_maps = []
    xTb, padbb, padkb = [], [], []
    for b in range(B):
        xTb.append(np.ascontiguousarray(x[b].T).astype(bf))
        pad = attention_mask[b].astype(np.float32)
        padbb.append(np.ascontiguousarray(
            np.broadcast_to(pad[None, :], (128, T))).astype(bf))
        pk = np.broadcast_to(
            pad.reshape(16, 128, 1), (16, 128, 64))
        padkb.append(np.ascontiguousarray(
            pk.transpose(1, 0, 2).reshape(128, 16 * 64)).astype(bf))
    for c in range(NCORES):
        b, g2 = c // 2, c % 2
        wq_blocks, bq_blocks, wo_blocks = [], [], []
        for hp in range(G):
            h0 = 8 * g2 + 2 * hp
            rows2 = np.r_[64 * h0:64 * h0 + 128]
            for ft in range(3):
                rows = ft * C + rows2
                wq_blocks.append(Wqkv[rows, :])
                bq_blocks.append(bqkv[rows] * (0.125 if ft == 0 else 1.0))
            wo_blocks.append(Wout[:, rows2].T)
        wq_c = np.ascontiguousarray(
            np.concatenate(wq_blocks, 0).T).astype(bf)
        bq_c = np.ascontiguousarray(
            np.stack(bq_blocks, 1).astype(np.float32, copy=False))
        wo_c = np.ascontiguousarray(
            np.concatenate(wo_blocks, 0).astype(np.float32, copy=False))
        in_maps.append({"xT": xTb[b], "wq": wq_c, "bq": bq_c, "wo": wo_c,
                       "padb": padbb[b], "padk": padkb[b]})
    return in_maps


def kernel(x, attention_mask, Wqkv, bqkv, Wout, _trace=False):
    x = np.asarray(x)
    attention_mask = np.asarray(attention_mask)
    Wqkv = np.asarray(Wqkv)
    bqkv = np.asarray(bqkv)
    Wout = np.asarray(Wout)
    ones = bool(np.all(attention_mask == 1))
    key = ("nc_ones" if ones else "nc")
    if key not in _cached:
        _cached[key] = build(ones_mask=ones)
    nc = _cached[key]
    in_maps = make_in_maps(x, attention_mask, Wqkv, bqkv, Wout)
    res = bass_utils.run_bass_kernel_spmd(
        nc, in_maps, core_ids=list(range(NCORES)), trace=_trace)
    out = np.empty((B, T, C), np.float32)
    for b in range(B):
        acc = res.results[2 * b]["outT"].astype(np.float32)
        acc += res.results[2 * b + 1]["outT"].astype(np.float32)
        out[b] = acc.T
    if _trace:
        _cached["last_result"] = res
    return out
